# revision 8
# baseline (speedup 1.0000x reference)
"""AttentiveManifoldMixer Trainium2 kernel (8-core data parallel over batch).

Math: with W3[c,i,j] = conv_w[c*64+i, j], B = conv_b.reshape(C, C),
  s[b]       = sigmoid(fc2 @ relu(fc1 @ mean_hw(x[b])))
  out[b,c,p] = sum_{i,j} W3[c,i,j] * s[b,j] * x[b,i,p] * x[b,j,p]
               + sum_i B[c,i] * x[b,i,p]

The quadratic form is symmetrized over unordered channel pairs grouped by
cyclic diagonal offset d: chunk m = 3k+l holds lanes q = 64*qhi + qlo with
  i = (qlo + a_k + qhi) % 64,   a_k = (64 - 6k) % 64
  j = (qlo + 2l + 2*qhi) % 64
so d = j - i = 6k + 2l + qhi covers 0..33 over 17 chunks (d=32/33 lanes
duplicate at higher mult).  Per-batch weight (W3[c,i,j]s_j + W3[c,j,i]s_i)
/ mult is folded on device.

Feature operands are windows of a row-doubled bf16 copy of x staged in DRAM
(xb2d, 130 rows).  All 9 window tiles (6 A + 3 B) live in one packed SBUF
tensor V[128, 9, P]; each qhi-half of each tile family loads with ONE DMA
(the tile index is an affine row stride in DRAM), so a column half needs
just 3 writes + 6 reads.  GEMM: 17 bf16 matmuls per 512-pixel subtile,
column-tiled 2x across the PE array (tile_position (0,0)/(0,64), two
subtiles share each PSUM bank), plus a (64,64)-mode float32r conv_b matmul.
The SE sigmoid vector is expanded to per-lane columns with 4 small DRAM
gathers; the weight fold runs on ACT with the final add on GPSIMD.
"""
import sys

sys.path.insert(0, "/opt/trn_rl_repo")

import numpy as np
import ml_dtypes

B, C, H, W = 8, 64, 64, 64
P = H * W                  # 4096 pixels per sample
MID = C // 4
NCHUNK = 17                # feature chunks
NA, NB = 6, 3              # A/B window tiles; chunk m = 3*(m//3) + m%3
NSUB = 512                 # matmul free-dim subtile / psum bank columns
NSPLIT = 2                 # column halves
HALF = P // NSPLIT
NBANK = HALF // (2 * NSUB)  # psum banks per half (2 subtiles per bank)
N_CORES = 8
A_VALS = [64, 58, 52, 46, 40, 34]   # a_k row offset, k=0..5 (64 == 0 mod 64)
NT = NA + NB               # packed window tiles in V

_CACHE = {}


def _lane_maps():
    """Per-lane (i, j, mult): chunk m = 3k+l, lane q = 64*qhi + qlo:
    i = (qlo + a_k + qhi) % 64,  j = (qlo + 2l + 2*qhi) % 64."""
    i_idx = np.zeros((NCHUNK, 128), np.int64)
    j_idx = np.zeros((NCHUNK, 128), np.int64)
    for m in range(NCHUNK):
        k, l = divmod(m, 3)
        for q in range(128):
            qhi, qlo = divmod(q, 64)
            i_idx[m, q] = (qlo + A_VALS[k] + qhi) % 64
            j_idx[m, q] = (qlo + 2 * l + 2 * qhi) % 64
    lo = np.minimum(i_idx, j_idx)
    hi = np.maximum(i_idx, j_idx)
    key = lo * 64 + hi
    _, inv, counts = np.unique(key, return_inverse=True, return_counts=True)
    mult = counts[inv].reshape(key.shape).astype(np.float32)
    return i_idx, j_idx, mult


def _host_weights(conv_w, fc1_w, fc2_w):
    """Pre-gather conv_w into per-lane arrays a1/a2 of shape (128, 17, 64):
    [lane q, chunk m, out-channel c], bf16."""
    w3 = conv_w.reshape(C, C, C)  # [c, i, j]
    i_idx, j_idx, mult = _lane_maps()
    a1 = np.transpose(w3[:, i_idx, j_idx], (2, 1, 0)) / mult.T[:, :, None]
    a2 = np.transpose(w3[:, j_idx, i_idx], (2, 1, 0)) / mult.T[:, :, None]
    diag = (i_idx == j_idx).T  # [q, m]
    a2[diag] = 0.0
    fc1t = (fc1_w.T / float(P)).copy()   # (64, 16): folds the 1/HW of the mean
    fc2t = fc2_w.T.copy()                # (16, 64)
    return (np.ascontiguousarray(a1, ml_dtypes.bfloat16),
            np.ascontiguousarray(a2, ml_dtypes.bfloat16), fc1t, fc2t)


def _host_idb(conv_b):
    """conv_b as a K=128-padded bf16 lhsT [B.T; 0] for the residual matmul
    (rhs is the b=0 window tile whose lower 64 rows are x)."""
    bt = np.asarray(conv_b, np.float32).reshape(C, C).T
    idb = np.zeros((128, C), np.float32)
    idb[0:C] = bt
    return np.ascontiguousarray(idb, ml_dtypes.bfloat16)


def _build_program(niter=None):
    """Build the kernel program; with niter, wrap the body in an on-device
    For_i repeat loop (timing variant)."""
    import contextlib

    import concourse.bacc as bacc
    import concourse.bass as bass
    from concourse import mybir
    from concourse.tile import TileContext

    nc = bacc.Bacc("TRN2", target_bir_lowering=False, debug=False)
    dt = mybir.dt

    x_d = nc.dram_tensor("x", [C, P], dt.float32r, kind="ExternalInput")
    a1_d = nc.dram_tensor("a1", [128, NCHUNK, C], dt.bfloat16, kind="ExternalInput")
    a2_d = nc.dram_tensor("a2", [128, NCHUNK, C], dt.bfloat16, kind="ExternalInput")
    f1_d = nc.dram_tensor("fc1t", [C, MID], dt.float32, kind="ExternalInput")
    f2_d = nc.dram_tensor("fc2t", [MID, C], dt.float32, kind="ExternalInput")
    id_d = nc.dram_tensor("ident", [128, C], dt.bfloat16, kind="ExternalInput")
    out_d = nc.dram_tensor("out", [C, P], dt.float32, kind="ExternalOutput")

    with TileContext(nc) as tc:
        with tc.tile_pool(name="single", bufs=1) as single, \
             tc.tile_pool(name="dram", bufs=1, space="DRAM") as dpool, \
             tc.tile_pool(name="feat", bufs=6) as featp, \
             tc.tile_pool(name="outs", bufs=4) as outsp, \
             tc.tile_pool(name="psum", bufs=8, space="PSUM") as psum, \
             (tc.For_i(0, niter, 1,
                       hint_engines=(mybir.EngineType.PE,
                                     mybir.EngineType.DVE,
                                     mybir.EngineType.SP,
                                     mybir.EngineType.Pool,
                                     mybir.EngineType.Activation))
              if niter else contextlib.nullcontext()):

            hsls = [slice(i * HALF, (i + 1) * HALF) for i in range(NSPLIT)]
            # Two HWDGE queues: nc.sync (SP) carries the staging writes +
            # window loads + output, nc.scalar (Activation) carries x/weights
            # and the SE-expansion gathers.
            xf = single.tile([C, P], dt.float32r)
            for hsl in hsls:
                nc.scalar.dma_start(out=xf[:, hsl], in_=x_d.ap()[:, hsl])
            a1s = single.tile([128, NCHUNK, C], dt.bfloat16)
            nc.scalar.dma_start(out=a1s, in_=a1_d.ap())
            a2s = single.tile([128, NCHUNK, C], dt.bfloat16)
            nc.scalar.dma_start(out=a2s, in_=a2_d.ap())
            f1s = single.tile([C, MID], dt.float32)
            nc.scalar.dma_start(out=f1s, in_=f1_d.ap())
            f2s = single.tile([MID, C], dt.float32)
            nc.scalar.dma_start(out=f2s, in_=f2_d.ap())
            ids = single.tile([128, C], dt.bfloat16)
            nc.scalar.dma_start(out=ids, in_=id_d.ap())

            # ---- prestage: cast x -> bf16 (+ per-half channel sums), stage
            # a row-doubled copy in DRAM, load the 9 window tiles into V.
            xb = single.tile([C, P], dt.bfloat16)
            sums_h = [single.tile([C, 1], dt.float32, name=f"sums{h}")
                      for h in range(NSPLIT)]
            xb2d = dpool.tile([130, P], dt.bfloat16)
            V = single.tile([128, NT, P], dt.bfloat16)
            VP = NT * P  # V per-partition pitch (elements)

            def vwin(part0, t0, nt_, hsl):
                return bass.AP(tensor=V.tensor,
                               offset=V.offset + part0 * VP + t0 * P + hsl.start,
                               ap=[[VP, 64], [P, nt_], [1, HALF]])

            for h, hsl in enumerate(hsls):
                nc.scalar.activation(xb[:, hsl], xf[:, hsl],
                                     mybir.ActivationFunctionType.Copy,
                                     accum_out=sums_h[h])
                nc.sync.dma_start(out=xb2d[0:C, hsl], in_=xb[:, hsl])
                nc.sync.dma_start(out=xb2d[C:2 * C, hsl], in_=xb[:, hsl])
                nc.sync.dma_start(out=xb2d[2 * C:2 * C + 2, hsl],
                                  in_=xb[0:2, hsl])

                def xwin(row0, tstep, nt_, hsl):
                    return bass.AP(
                        tensor=xb2d.tensor,
                        offset=xb2d.offset + row0 * P + hsl.start,
                        ap=[[P, 64], [tstep * P, nt_], [1, HALF]])

                # A-family slot t=0..5 holds a = 34+6t (k = 5-t); B-family
                # slot 6+l holds b = 2l.  One DMA per (family, qhi); slot 5
                # (k=0) loads first with B so chunks 0-2 unblock the DVE.
                for qhi in range(2):
                    nc.sync.dma_start(out=vwin(64 * qhi, 5, 1, hsl),
                                      in_=xwin(64 + qhi, 6, 1, hsl))
                    nc.sync.dma_start(out=vwin(64 * qhi, NA, NB, hsl),
                                      in_=xwin(2 * qhi, 2, NB, hsl))
                for qhi in range(2):
                    nc.sync.dma_start(out=vwin(64 * qhi, 0, NA - 1, hsl),
                                      in_=xwin(34 + qhi, 6, NA - 1, hsl))

            # ---- SE path: s = sigmoid(fc2t.T @ relu(fc1t.T @ sums)) ----
            ps1 = psum.tile([MID, 1], dt.float32, tag="acc")
            for h in range(NSPLIT):
                nc.tensor.matmul(ps1, f1s, sums_h[h], start=(h == 0),
                                 stop=(h == NSPLIT - 1))
            y1 = single.tile([MID, 1], dt.float32)
            nc.scalar.activation(y1, ps1, mybir.ActivationFunctionType.Relu)
            ps2 = psum.tile([C, 1], dt.float32, tag="acc")
            nc.tensor.matmul(ps2, f2s, y1, start=True, stop=True)
            svec = single.tile([C, 1], dt.float32)
            nc.scalar.activation(svec, ps2, mybir.ActivationFunctionType.Sigmoid)

            # s -> DRAM (s_int = [s; s; s[0:2]]) for the expansion gathers
            s_int = dpool.tile([130], dt.float32)
            nc.scalar.dma_start(out=s_int[0:C][:, None], in_=svec)
            nc.scalar.dma_start(out=s_int[C:2 * C][:, None], in_=svec)
            nc.scalar.dma_start(out=s_int[2 * C:2 * C + 2][:, None],
                                in_=svec[0:2, :])

            # gathers: s1b[q, l] = s[j(l, qhi, qlo)] = s_int[qlo + 2l + 2qhi]
            #          s2b[q, K] = s[i(k=5-K, ..)] = s_int[qlo + 34+6K + qhi]
            s1b = single.tile([128, NB], dt.float32)
            s2b = single.tile([128, NA], dt.float32)
            for qhi in range(2):
                nc.scalar.dma_start(
                    out=s1b[64 * qhi:64 * qhi + 64, :],
                    in_=bass.AP(tensor=s_int.tensor,
                                offset=s_int.offset + 2 * qhi,
                                ap=[[1, 64], [2, NB]]))
                nc.scalar.dma_start(
                    out=s2b[64 * qhi:64 * qhi + 64, :],
                    in_=bass.AP(tensor=s_int.tensor,
                                offset=s_int.offset + 34 + qhi,
                                ap=[[1, 64], [6, NA]]))

            # ---- fold s into weights: wc = a1*s[j] + a2*s[i] (bf16) ----
            # s1b col l serves chunks m = l (mod 3); s2b col K=5-k serves
            # chunks 3k..3k+2.  ACT muls + one GPSIMD add.
            wc = single.tile([128, NCHUNK, C], dt.bfloat16)
            t1 = single.tile([128, NCHUNK, C], dt.float32)
            t2 = single.tile([128, NCHUNK, C], dt.float32)
            for l in range(NB):
                nc.scalar.mul(t1[:, l::3, :], a1s[:, l::3, :], s1b[:, l:l + 1])
            for k in range(NA):
                ms = slice(3 * k, min(3 * k + 3, NCHUNK))
                nc.scalar.mul(t2[:, ms, :], a2s[:, ms, :],
                              s2b[:, 5 - k:6 - k])
            nc.gpsimd.tensor_add(
                wc.rearrange("p a b -> p (a b)"),
                t1.rearrange("p a b -> p (a b)"),
                t2.rearrange("p a b -> p (a b)"))

            # ---- main sweep: per half, 17 feature TTs feed a column-tiled
            # GEMM (two 512-col subtiles run concurrently per PSUM bank).
            for h, hsl in enumerate(hsls):
                banks = [psum.tile([128, NSUB], dt.float32, tag="acc",
                                   name=f"bank{h}_{j}") for j in range(NBANK)]
                for m in range(NCHUNK):
                    k, l = divmod(m, 3)
                    f = featp.tile([128, HALF], dt.bfloat16, tag="f")
                    nc.vector.tensor_mul(
                        f,
                        bass.AP(tensor=V.tensor,
                                offset=V.offset + (5 - k) * P + hsl.start,
                                ap=[[VP, 128], [1, HALF]]),
                        bass.AP(tensor=V.tensor,
                                offset=V.offset + (NA + l) * P + hsl.start,
                                ap=[[VP, 128], [1, HALF]]))
                    for j in range(NBANK):
                        for ct in range(2):
                            sub = slice((2 * j + ct) * NSUB,
                                        (2 * j + ct + 1) * NSUB)
                            nc.tensor.matmul(
                                banks[j][64 * ct:64 * ct + 64, :],
                                wc[:, m, :], f[:, sub],
                                start=(m == 0), stop=False,
                                tile_position=(0, 64 * ct))
                # conv_b term: += [B.T; 0].T @ V[b=0 slot] = B @ x (bf16,
                # same (128,64) tile mode as the chunk matmuls)
                for j in range(NBANK):
                    for ct in range(2):
                        col = h * HALF + (2 * j + ct) * NSUB
                        nc.tensor.matmul(
                            banks[j][64 * ct:64 * ct + 64, :],
                            ids,
                            bass.AP(tensor=V.tensor,
                                    offset=V.offset + NA * P + col,
                                    ap=[[VP, 128], [1, NSUB]]),
                            start=False, stop=True,
                            tile_position=(0, 64 * ct))
                for j in range(NBANK):
                    ot = outsp.tile([128, NSUB], dt.float32, tag="o")
                    nc.scalar.copy(ot, banks[j])
                    for ct in range(2):
                        col = h * HALF + (2 * j + ct) * NSUB
                        nc.sync.dma_start(
                            out=out_d.ap()[:, col:col + NSUB],
                            in_=ot[64 * ct:64 * ct + 64, :])

    nc.compile()
    return nc


def _get_program(niter=None):
    key = ("nc", niter)
    if key not in _CACHE:
        _CACHE[key] = _build_program(niter)
    return _CACHE[key]


def kernel(x, fc1_w, fc2_w, conv_w, conv_b):
    from concourse.bass_utils import run_bass_kernel_spmd

    x = np.asarray(x, np.float32)
    a1, a2, fc1t, fc2t = _host_weights(
        np.asarray(conv_w, np.float32), np.asarray(fc1_w, np.float32),
        np.asarray(fc2_w, np.float32))
    # conv_b contributes sum_i B[c,i]*x_i with B = conv_b.reshape(C, C); the
    # "residual" matmul realizes it with lhsT = [B.T; 0] (identity-init -> +x).
    ident = _host_idb(conv_b)
    nc = _get_program()
    in_maps = []
    for b in range(N_CORES):
        in_maps.append({
            "x": np.ascontiguousarray(x[b].reshape(C, P)),
            "a1": a1, "a2": a2, "fc1t": fc1t, "fc2t": fc2t, "ident": ident,
        })
    res = run_bass_kernel_spmd(nc, in_maps, core_ids=list(range(N_CORES)))
    out = np.stack([res.results[b]["out"].reshape(C, H, W)
                    for b in range(N_CORES)], axis=0)
    return out.astype(np.float32)


# revision 10
# speedup vs baseline: 1.1363x; 1.1363x over previous
"""AttentiveManifoldMixer Trainium2 kernel (8-core data parallel over batch).

Math: with W3[c,i,j] = conv_w[c*64+i, j], B = conv_b.reshape(C, C),
  s[b]       = sigmoid(fc2 @ relu(fc1 @ mean_hw(x[b])))
  out[b,c,p] = sum_{i,j} W3[c,i,j] * s[b,j] * x[b,i,p] * x[b,j,p]
               + sum_i B[c,i] * x[b,i,p]

The quadratic form is symmetrized over unordered channel pairs grouped by
cyclic diagonal offset d: chunk m = 3k+l holds lanes q = 64*qhi + qlo with
  i = (qlo + a_k + qhi) % 64,   a_k = (64 - 6k) % 64
  j = (qlo + 2l + 2*qhi) % 64
so d = j - i = 6k + 2l + qhi covers 0..33 over 17 chunks (d=32/33 lanes
duplicate at higher mult).  Per-batch weight (W3[c,i,j]s_j + W3[c,j,i]s_i)
/ mult is folded on device.

Feature operands are windows of a row-doubled bf16 copy of x staged in DRAM
(xb2d, 130 rows).  All 9 window tiles (6 A + 3 B) live in one packed SBUF
tensor V[128, 9, P]; each qhi-half of each tile family loads with ONE DMA
(the tile index is an affine row stride in DRAM), so a column half needs
just 3 writes + 6 reads.  GEMM: 17 bf16 matmuls per 512-pixel subtile,
column-tiled 2x across the PE array (tile_position (0,0)/(0,64), two
subtiles share each PSUM bank), plus a (64,64)-mode float32r conv_b matmul.
The SE sigmoid vector is expanded to per-lane columns with 4 small DRAM
gathers; the weight fold runs on ACT with the final add on GPSIMD.
"""
import sys

sys.path.insert(0, "/opt/trn_rl_repo")

import numpy as np
import ml_dtypes

B, C, H, W = 8, 64, 64, 64
P = H * W                  # 4096 pixels per sample
MID = C // 4
NCHUNK = 17                # feature chunks
NA, NB = 6, 3              # A/B window tiles; chunk m = 3*(m//3) + m%3
NSUB = 512                 # matmul free-dim subtile / psum bank columns
NSPLIT = 2                 # column halves
HALF = P // NSPLIT
NBANK = HALF // (2 * NSUB)  # psum banks per half (2 subtiles per bank)
N_CORES = 8
A_VALS = [64, 58, 52, 46, 40, 34]   # a_k row offset, k=0..5 (64 == 0 mod 64)
NT = NA + NB               # packed window tiles in V

_CACHE = {}


def _lane_maps():
    """Per-lane (i, j, mult): chunk m = 3k+l, lane q = 64*qhi + qlo:
    i = (qlo + a_k + qhi) % 64,  j = (qlo + 2l + 2*qhi) % 64."""
    i_idx = np.zeros((NCHUNK, 128), np.int64)
    j_idx = np.zeros((NCHUNK, 128), np.int64)
    for m in range(NCHUNK):
        k, l = divmod(m, 3)
        for q in range(128):
            qhi, qlo = divmod(q, 64)
            i_idx[m, q] = (qlo + A_VALS[k] + qhi) % 64
            j_idx[m, q] = (qlo + 2 * l + 2 * qhi) % 64
    lo = np.minimum(i_idx, j_idx)
    hi = np.maximum(i_idx, j_idx)
    key = lo * 64 + hi
    _, inv, counts = np.unique(key, return_inverse=True, return_counts=True)
    mult = counts[inv].reshape(key.shape).astype(np.float32)
    return i_idx, j_idx, mult


def _host_weights(conv_w, fc1_w, fc2_w):
    """Pre-gather conv_w into per-lane arrays a1/a2 of shape (128, 17, 64):
    [lane q, chunk m, out-channel c], bf16."""
    w3 = conv_w.reshape(C, C, C)  # [c, i, j]
    i_idx, j_idx, mult = _lane_maps()
    a1 = np.transpose(w3[:, i_idx, j_idx], (2, 1, 0)) / mult.T[:, :, None]
    a2 = np.transpose(w3[:, j_idx, i_idx], (2, 1, 0)) / mult.T[:, :, None]
    diag = (i_idx == j_idx).T  # [q, m]
    a2[diag] = 0.0
    fc1t = (fc1_w.T / float(P)).copy()   # (64, 16): folds the 1/HW of the mean
    fc2t = fc2_w.T.copy()                # (16, 64)
    return (np.ascontiguousarray(a1, ml_dtypes.bfloat16),
            np.ascontiguousarray(a2, ml_dtypes.bfloat16), fc1t, fc2t)


def _host_idb(conv_b):
    """conv_b as a K=128-padded bf16 lhsT [B.T; 0] for the residual matmul
    (rhs is the b=0 window tile whose lower 64 rows are x)."""
    bt = np.asarray(conv_b, np.float32).reshape(C, C).T
    idb = np.zeros((128, C), np.float32)
    idb[0:C] = bt
    return np.ascontiguousarray(idb, ml_dtypes.bfloat16)


def _build_program(niter=None):
    """Build the kernel program; with niter, wrap the body in an on-device
    For_i repeat loop (timing variant)."""
    import contextlib

    import concourse.bacc as bacc
    import concourse.bass as bass
    from concourse import mybir
    from concourse.tile import TileContext

    nc = bacc.Bacc("TRN2", target_bir_lowering=False, debug=False)
    dt = mybir.dt

    x_d = nc.dram_tensor("x", [C, P], dt.float32r, kind="ExternalInput")
    a1_d = nc.dram_tensor("a1", [128, NCHUNK, C], dt.bfloat16, kind="ExternalInput")
    a2_d = nc.dram_tensor("a2", [128, NCHUNK, C], dt.bfloat16, kind="ExternalInput")
    f1_d = nc.dram_tensor("fc1t", [C, MID], dt.float32, kind="ExternalInput")
    f2_d = nc.dram_tensor("fc2t", [MID, C], dt.float32, kind="ExternalInput")
    id_d = nc.dram_tensor("ident", [128, C], dt.bfloat16, kind="ExternalInput")
    out_d = nc.dram_tensor("out", [C, P], dt.float32, kind="ExternalOutput")

    with TileContext(nc) as tc:
        with tc.tile_pool(name="single", bufs=1) as single, \
             tc.tile_pool(name="dram", bufs=1, space="DRAM") as dpool, \
             tc.tile_pool(name="feat", bufs=6) as featp, \
             tc.tile_pool(name="outs", bufs=4) as outsp, \
             tc.tile_pool(name="psum", bufs=8, space="PSUM") as psum, \
             (tc.For_i(0, niter, 1,
                       hint_engines=(mybir.EngineType.PE,
                                     mybir.EngineType.DVE,
                                     mybir.EngineType.SP,
                                     mybir.EngineType.Pool,
                                     mybir.EngineType.Activation))
              if niter else contextlib.nullcontext()):

            hsls = [slice(i * HALF, (i + 1) * HALF) for i in range(NSPLIT)]
            # Two HWDGE queues: nc.sync (SP) carries the staging writes +
            # window loads + output, nc.scalar (Activation) carries x/weights
            # and the SE-expansion gathers.
            xf = single.tile([C, P], dt.float32r)
            for hsl in hsls:
                nc.scalar.dma_start(out=xf[:, hsl], in_=x_d.ap()[:, hsl])
            a1s = single.tile([128, NCHUNK, C], dt.bfloat16)
            nc.scalar.dma_start(out=a1s, in_=a1_d.ap())
            a2s = single.tile([128, NCHUNK, C], dt.bfloat16)
            nc.scalar.dma_start(out=a2s, in_=a2_d.ap())
            f1s = single.tile([C, MID], dt.float32)
            nc.scalar.dma_start(out=f1s, in_=f1_d.ap())
            f2s = single.tile([MID, C], dt.float32)
            nc.scalar.dma_start(out=f2s, in_=f2_d.ap())
            ids = single.tile([128, C], dt.bfloat16)
            nc.scalar.dma_start(out=ids, in_=id_d.ap())

            # ---- prestage: cast x -> [x; x] bf16 in SBUF (X2, + per-half
            # channel sums on the low cast), then build the 9 window tiles
            # with direct SBUF->SBUF partition-window DMAs (no DRAM staging).
            # A_k window (a_k, a_k+1), a = [0, 58, 52, 46, 40, 34];
            # B_l window (2l, 2l+2).
            sums_h = [single.tile([C, 1], dt.float32, name=f"sums{h}")
                      for h in range(NSPLIT)]
            X2 = single.tile([128, P], dt.bfloat16)
            V = single.tile([128, NT, P], dt.bfloat16)
            VP = NT * P  # V per-partition pitch (elements)
            a_low = [0, 58, 52, 46, 40, 34]

            def vdst(qhi, t, hsl):
                return bass.AP(tensor=V.tensor,
                               offset=V.offset + 64 * qhi * VP + t * P
                               + hsl.start,
                               ap=[[VP, 64], [1, HALF]])

            def xwin(row0, hsl):
                return bass.AP(tensor=X2.tensor,
                               offset=X2.offset + row0 * P + hsl.start,
                               ap=[[P, 64], [1, HALF]])

            for h, hsl in enumerate(hsls):
                nc.scalar.activation(X2[0:C, hsl], xf[:, hsl],
                                     mybir.ActivationFunctionType.Copy,
                                     accum_out=sums_h[h])
                nc.scalar.activation(X2[C:128, hsl], xf[:, hsl],
                                     mybir.ActivationFunctionType.Copy)
                # copy order tracks first use: chunk m = 3k+l needs A-slot
                # k = m//3 and B-slot 6 + m%3.
                order = [(0, a_low[0], 0)] + \
                    [(NA + l, 2 * l, 2 * l) for l in range(NB)] + \
                    [(k, a_low[k], a_low[k]) for k in range(1, NA)]
                for t, wlo, whi_base in order:
                    whi = whi_base + (1 if t < NA else 2)
                    nc.sync.dma_start(out=vdst(0, t, hsl),
                                      in_=xwin(wlo, hsl))
                    nc.sync.dma_start(out=vdst(1, t, hsl),
                                      in_=xwin(whi, hsl))

            # ---- SE path: s = sigmoid(fc2t.T @ relu(fc1t.T @ sums)) ----
            ps1 = psum.tile([MID, 1], dt.float32, tag="acc")
            for h in range(NSPLIT):
                nc.tensor.matmul(ps1, f1s, sums_h[h], start=(h == 0),
                                 stop=(h == NSPLIT - 1))
            y1 = single.tile([MID, 1], dt.float32)
            nc.scalar.activation(y1, ps1, mybir.ActivationFunctionType.Relu)
            ps2 = psum.tile([C, 1], dt.float32, tag="acc")
            nc.tensor.matmul(ps2, f2s, y1, start=True, stop=True)
            svec = single.tile([C, 1], dt.float32)
            nc.scalar.activation(svec, ps2, mybir.ActivationFunctionType.Sigmoid)

            # s -> DRAM (s_int = [s; s; s[0:2]]) for the expansion gathers
            s_int = dpool.tile([130], dt.float32)
            nc.scalar.dma_start(out=s_int[0:C][:, None], in_=svec)
            nc.scalar.dma_start(out=s_int[C:2 * C][:, None], in_=svec)
            nc.scalar.dma_start(out=s_int[2 * C:2 * C + 2][:, None],
                                in_=svec[0:2, :])

            # gathers: s1b[q, l] = s[j(l, qhi, qlo)] = s_int[qlo + 2l + 2qhi]
            #          s2b[q, K] = s[i(k=5-K, ..)] = s_int[qlo + 34+6K + qhi]
            s1b = single.tile([128, NB], dt.float32)
            s2b = single.tile([128, NA], dt.float32)
            for qhi in range(2):
                nc.scalar.dma_start(
                    out=s1b[64 * qhi:64 * qhi + 64, :],
                    in_=bass.AP(tensor=s_int.tensor,
                                offset=s_int.offset + 2 * qhi,
                                ap=[[1, 64], [2, NB]]))
                nc.scalar.dma_start(
                    out=s2b[64 * qhi:64 * qhi + 64, :],
                    in_=bass.AP(tensor=s_int.tensor,
                                offset=s_int.offset + 34 + qhi,
                                ap=[[1, 64], [6, NA]]))

            # ---- fold s into weights: wc = a1*s[j] + a2*s[i] (bf16) ----
            # s1b col l serves chunks m = l (mod 3); s2b col K=5-k serves
            # chunks 3k..3k+2.  ACT muls + one GPSIMD add.
            wc = single.tile([128, NCHUNK, C], dt.bfloat16)
            t1 = single.tile([128, NCHUNK, C], dt.float32)
            t2 = single.tile([128, NCHUNK, C], dt.float32)
            for l in range(NB):
                nc.scalar.mul(t1[:, l::3, :], a1s[:, l::3, :], s1b[:, l:l + 1])
            for k in range(NA):
                ms = slice(3 * k, min(3 * k + 3, NCHUNK))
                nc.scalar.mul(t2[:, ms, :], a2s[:, ms, :],
                              s2b[:, 5 - k:6 - k])
            nc.gpsimd.tensor_add(
                wc.rearrange("p a b -> p (a b)"),
                t1.rearrange("p a b -> p (a b)"),
                t2.rearrange("p a b -> p (a b)"))

            # ---- main sweep: per half, 17 feature TTs feed a column-tiled
            # GEMM (two 512-col subtiles run concurrently per PSUM bank).
            NSH = HALF // NSUB
            for h, hsl in enumerate(hsls):
                banks = [psum.tile([C, NSUB], dt.float32, tag="acc",
                                   name=f"bank{h}_{j}") for j in range(NSH)]
                for m in range(NCHUNK):
                    k, l = divmod(m, 3)
                    f = featp.tile([128, HALF], dt.bfloat16, tag="f")
                    nc.vector.tensor_mul(
                        f,
                        bass.AP(tensor=V.tensor,
                                offset=V.offset + k * P + hsl.start,
                                ap=[[VP, 128], [1, HALF]]),
                        bass.AP(tensor=V.tensor,
                                offset=V.offset + (NA + l) * P + hsl.start,
                                ap=[[VP, 128], [1, HALF]]))
                    for j in range(NSH):
                        nc.tensor.matmul(banks[j], wc[:, m, :],
                                         f[:, j * NSUB:(j + 1) * NSUB],
                                         start=(m == 0), stop=False)
                # conv_b term: += [B.T; 0].T @ V[b=0 slot] = B @ x (bf16)
                for j in range(NSH):
                    col = h * HALF + j * NSUB
                    nc.tensor.matmul(
                        banks[j], ids,
                        bass.AP(tensor=V.tensor,
                                offset=V.offset + NA * P + col,
                                ap=[[VP, 128], [1, NSUB]]),
                        start=False, stop=True)
                for j in range(NSH):
                    col = h * HALF + j * NSUB
                    ot = outsp.tile([C, NSUB], dt.float32, tag="o")
                    nc.scalar.copy(ot, banks[j])
                    nc.sync.dma_start(out=out_d.ap()[:, col:col + NSUB],
                                      in_=ot)

    nc.compile()
    return nc


def _get_program(niter=None):
    key = ("nc", niter)
    if key not in _CACHE:
        _CACHE[key] = _build_program(niter)
    return _CACHE[key]


def kernel(x, fc1_w, fc2_w, conv_w, conv_b):
    from concourse.bass_utils import run_bass_kernel_spmd

    x = np.asarray(x, np.float32)
    a1, a2, fc1t, fc2t = _host_weights(
        np.asarray(conv_w, np.float32), np.asarray(fc1_w, np.float32),
        np.asarray(fc2_w, np.float32))
    # conv_b contributes sum_i B[c,i]*x_i with B = conv_b.reshape(C, C); the
    # "residual" matmul realizes it with lhsT = [B.T; 0] (identity-init -> +x).
    ident = _host_idb(conv_b)
    nc = _get_program()
    in_maps = []
    for b in range(N_CORES):
        in_maps.append({
            "x": np.ascontiguousarray(x[b].reshape(C, P)),
            "a1": a1, "a2": a2, "fc1t": fc1t, "fc2t": fc2t, "ident": ident,
        })
    res = run_bass_kernel_spmd(nc, in_maps, core_ids=list(range(N_CORES)))
    out = np.stack([res.results[b]["out"].reshape(C, H, W)
                    for b in range(N_CORES)], axis=0)
    return out.astype(np.float32)
